# revision 1
# baseline (speedup 1.0000x reference)
"""Trainium2 Bass kernel for nn_NodeRNN (masked single-step LSTM over N nodes).

Strategy: pure data parallel over the node dim N across 8 cores. All per-node
tensors are staged FEATURE-MAJOR (transposed on host) so that every DMA is
contiguous (4KB runs) and every matmul gets its contraction dim on partitions
with no on-device transposes. Outputs come back feature-major and are
transposed back on host.

Per 1024-node tile (feature-major [features, nodes], two 512-node matmul
subtiles per PSUM bank pair):
  x.T   = [relu(W_pos @ xv.T + b_pos); relu(W_hid @ X.T + b_hid)]  (PE + ACT)
  gates = W_ih @ x.T + W_hh @ hv.T (+ biases via ACT)              (PE)
  i,f,o = sigmoid, g = tanh                                        (ACT)
  c_new = f*cv + i*g ; h_new = o*tanh(c_new)                       (DVE)
  inactive rows get old hv/cv copied back over h_new/c_new         (DVE + GPSIMD mask bcast)
Matmuls run as float32r (1 col/cycle, ~1e-4 rel err) on f32 data.
Emission is software-pipelined (stage A of tile t+1 before stage B of tile t)
to keep the PE stream dense so the HAM clock stays warm.
"""
import sys

sys.path.insert(0, "/opt/trn_rl_repo")

import numpy as np

import concourse.bacc as bacc
import concourse.tile as tile
from concourse import mybir
from concourse.bass_utils import run_bass_kernel_spmd

f32 = mybir.dt.float32
f32r = mybir.dt.float32r
i32 = mybir.dt.int32
AF = mybir.ActivationFunctionType
ALU = mybir.AluOpType

N = 262144
NCORES = 8
NS = N // NCORES          # 32768 nodes per core
T = 1024                  # nodes per tile (DMA + elementwise granularity)
TS = 512                  # matmul subtile (PSUM bank = 512 f32)
NT = NS // T              # 32 tiles per core
EMBED = 64
EDGE_H = 256
NODE_H = 128
XF = 2 * EDGE_H           # 512 concat(hvv, Hv) features

# const block layout: [128, CF] f32, free-dim offsets
CO_WHID = 0               # 4 chunks x 128 cols; cols 64:128 of chunk c = W_hid.T chunk
CO_WIH = 512              # W_ih.T [128, 512]
CO_WHH = 1024             # W_hh.T [128, 512]
CO_BX = 1536              # concat(b_pos, b_hid) [128, 1]
CO_BG = 1537              # (b_ih + b_hh) as [128, 4], col j = gate chunk j
CO_WP = 1541              # W_pos.T rows 0:2, [2, 64]
CF = 1632

GATE_FUNCS = [AF.Sigmoid, AF.Sigmoid, AF.Tanh, AF.Sigmoid]  # i, f, g, o

_cached = {}


def build_nc():
    nc = bacc.Bacc(target_bir_lowering=False)
    xt_d = nc.dram_tensor("xt", [XF, NS], f32r, kind="ExternalInput")
    hc_d = nc.dram_tensor("hc", [2 * NODE_H, NS], f32r, kind="ExternalInput")
    aux_d = nc.dram_tensor("aux", [2, NS], f32r, kind="ExternalInput")
    mk_d = nc.dram_tensor("mk", [1, NS], f32r, kind="ExternalInput")
    cst_d = nc.dram_tensor("cst", [128, CF], f32r, kind="ExternalInput")
    out_d = nc.dram_tensor("hc_out", [2 * NODE_H, NS], f32, kind="ExternalOutput")

    xt_v = xt_d[:].rearrange("(c p) n -> p c n", p=128)    # [128, 4, NS]
    hc_v = hc_d[:].rearrange("(c p) n -> p c n", p=128)    # [128, 2, NS]
    out_v = out_d[:].rearrange("(c p) n -> p c n", p=128)  # [128, 2, NS]

    with tile.TileContext(nc) as tc:
        with (
            tc.tile_pool(name="const", bufs=1) as cpool,
            tc.tile_pool(name="xt", bufs=3) as xtp,
            tc.tile_pool(name="hc", bufs=3) as hcp,
            tc.tile_pool(name="aux", bufs=3) as auxp,
            tc.tile_pool(name="xsb", bufs=2) as xsbp,
            tc.tile_pool(name="msk", bufs=2) as mskp,
            tc.tile_pool(name="gact", bufs=5) as gactp,
            tc.tile_pool(name="tmp", bufs=2) as tmpp,
            tc.tile_pool(name="hcn", bufs=2) as hcnp,
            tc.tile_pool(name="ps_x", bufs=2, space="PSUM") as psx,
            tc.tile_pool(name="ps_g", bufs=2, space="PSUM") as psg,
        ):
            cst = cpool.tile([128, CF], f32r)
            nc.sync.dma_start(cst[:], cst_d[:])

            # warmup matmul absorbs the cst DMA wait on the PE
            warm = psx.tile([64, 256], f32, tag="x")
            nc.tensor.matmul(warm[:], cst[0:2, CO_WP:CO_WP + 64],
                             cst[0:2, 0:256], start=True, stop=True)

            stash = {}

            def stage_a(t):
                nsl = slice(t * T, (t + 1) * T)
                xt_t = xtp.tile([128, 4, T], f32r, tag="xt")
                nc.sync.dma_start(xt_t[:], xt_v[:, :, nsl])
                hc_t = hcp.tile([128, 2, T], f32r, tag="hc")
                nc.sync.dma_start(hc_t[:], hc_v[:, :, nsl])
                aux_t = auxp.tile([2, T], f32r, tag="aux")
                nc.sync.dma_start(aux_t[:], aux_d[:, nsl])
                mk_t = auxp.tile([1, T], f32r, tag="mk")
                nc.sync.dma_start(mk_t[:], mk_d[:, nsl])

                # inverted-mask broadcast on the (otherwise idle) GPSIMD
                m_sb = mskp.tile([128, T], f32, tag="m")
                nc.gpsimd.partition_broadcast(m_sb[:], mk_t[:].bitcast(f32))

                # x.T psum [128, 1024] (2 banks; each 512-subtile is one bank):
                # partitions 0:64 e_v, 64:128 a_v (zero-padded lhsT)
                x_ps = psx.tile([128, T], f32, tag="x")
                for k in range(T // TS):
                    ksl = slice(k * TS, (k + 1) * TS)
                    for c in range(4):
                        nc.tensor.matmul(
                            x_ps[:, ksl],
                            cst[:, CO_WHID + 128 * c:CO_WHID + 128 * (c + 1)],
                            xt_t[:, c, ksl], start=(c == 0), stop=False,
                            skip_group_check=True)
                    nc.tensor.matmul(x_ps[0:64, ksl], cst[0:2, CO_WP:CO_WP + 64],
                                     aux_t[0:2, ksl], start=False, stop=True,
                                     skip_group_check=True)

                # x = relu(x_ps + bias_x), rounded to f32r for the gate matmuls
                x_sb = xsbp.tile([128, T], f32r, tag="x_sb")
                nc.scalar.activation(x_sb[:], x_ps[:], AF.Relu,
                                     bias=cst[:, CO_BX:CO_BX + 1].bitcast(f32))
                stash[t] = (xt_t, hc_t, aux_t, m_sb, x_sb, nsl)

            def stage_b(t):
                xt_t, hc_t, aux_t, m_sb, x_sb, nsl = stash.pop(t)
                gact = []
                # per gate chunk j: g_ps_j = W_hh.T_j @ hv.T + W_ih.T_j @ x.T
                for j in range(4):
                    gp = psg.tile([128, T], f32, tag="g")
                    for k in range(T // TS):
                        ksl = slice(k * TS, (k + 1) * TS)
                        nc.tensor.matmul(
                            gp[:, ksl], cst[:, CO_WHH + 128 * j:CO_WHH + 128 * (j + 1)],
                            hc_t[:, 0, ksl], start=True, stop=False)
                        nc.tensor.matmul(
                            gp[:, ksl], cst[:, CO_WIH + 128 * j:CO_WIH + 128 * (j + 1)],
                            x_sb[:, ksl], start=False, stop=True)
                    ga = gactp.tile([128, T], f32, tag="ga")
                    gact.append(ga)
                    nc.scalar.activation(ga[:], gp[:], GATE_FUNCS[j],
                                         bias=cst[:, CO_BG + j:CO_BG + j + 1].bitcast(f32))
                i_s, f_s, g_t, o_s = gact

                hcn = hcnp.tile([128, 2, T], f32, tag="hcn")
                t1 = tmpp.tile([128, T], f32, tag="t1")
                t2 = tmpp.tile([128, T], f32, tag="t2")
                th = tmpp.tile([128, T], f32, tag="th")
                cv_ap = hc_t[:, 1, :].bitcast(f32)
                hv_ap = hc_t[:, 0, :].bitcast(f32)
                # t1 = (f + 0) * cv ; t2 = (i + 0) * g ; c_new = (t1 + 0) + t2
                nc.vector.scalar_tensor_tensor(t1[:], f_s[:], 0.0, cv_ap, ALU.add, ALU.mult)
                nc.vector.scalar_tensor_tensor(t2[:], i_s[:], 0.0, g_t[:], ALU.add, ALU.mult)
                nc.vector.scalar_tensor_tensor(hcn[:, 1, :], t1[:], 0.0, t2[:], ALU.add, ALU.add)
                nc.scalar.activation(th[:], hcn[:, 1, :], AF.Tanh)
                # h_new = (o + 0) * tanh(c_new)
                nc.vector.scalar_tensor_tensor(hcn[:, 0, :], o_s[:], 0.0, th[:], ALU.add, ALU.mult)

                # m_sb broadcasts the INVERTED mask: overwrite h_new/c_new with
                # the old hv/cv on inactive rows, then store. (hc_t stays
                # read-only so its only producer is the f32r DMA.)
                nc.vector.copy_predicated(hcn[:, 0, :], m_sb[:].bitcast(i32), hv_ap)
                nc.vector.copy_predicated(hcn[:, 1, :], m_sb[:].bitcast(i32), cv_ap)
                nc.sync.dma_start(out_v[:, :, nsl], hcn[:])

            for t in range(NT + 1):
                if t < NT:
                    stage_a(t)
                if t >= 1:
                    stage_b(t - 1)

    nc.finalize()
    return nc


def _stage_inputs(Hv_t, hvv_t, xv_t, hv_tm1, cv_tm1, ts_mask,
                  W_pos, b_pos, W_hid, b_hid, W_ih, b_ih, W_hh, b_hh):
    cst = np.zeros((128, CF), dtype=np.float32)
    whid_t = np.ascontiguousarray(W_hid.T)          # [512, 64]
    for c in range(4):
        cst[:, CO_WHID + 128 * c + 64:CO_WHID + 128 * (c + 1)] = whid_t[128 * c:128 * (c + 1)]
    cst[:, CO_WIH:CO_WIH + 512] = W_ih.T            # [128, 512]
    cst[:, CO_WHH:CO_WHH + 512] = W_hh.T
    cst[:, CO_BX] = np.concatenate([b_pos, b_hid])
    bg = b_ih + b_hh
    cst[:, CO_BG:CO_BG + 4] = bg.reshape(4, 128).T
    cst[0:2, CO_WP:CO_WP + 64] = W_pos.T

    # inverted mask: 1.0 where the node is INACTIVE (keeps old state)
    maskf = (ts_mask[:, 0] != 1).astype(np.float32)

    in_maps = []
    for s in range(NCORES):
        sl = slice(s * NS, (s + 1) * NS)
        xt = np.empty((XF, NS), dtype=np.float32)
        xt[0:EDGE_H] = hvv_t[sl].T
        xt[EDGE_H:] = Hv_t[sl].T
        hc = np.empty((2 * NODE_H, NS), dtype=np.float32)
        hc[0:NODE_H] = hv_tm1[sl].T
        hc[NODE_H:] = cv_tm1[sl].T
        aux = np.ascontiguousarray(xv_t[sl].T)
        mk = maskf[sl].reshape(1, NS)
        in_maps.append(dict(xt=xt, hc=hc, aux=aux, mk=mk, cst=cst))
    return in_maps


def run(inputs, trace=False):
    """Stage, run on 8 cores, unstage. Returns ((hv_t, cv_t), BassKernelResults)."""
    inputs = {k: np.asarray(v) for k, v in inputs.items()}
    in_maps = _stage_inputs(**inputs)
    if "nc" not in _cached:
        _cached["nc"] = build_nc()
    res = run_bass_kernel_spmd(_cached["nc"], in_maps, core_ids=list(range(NCORES)),
                               trace=trace)
    hv_out = np.empty((N, NODE_H), dtype=np.float32)
    cv_out = np.empty((N, NODE_H), dtype=np.float32)
    for s in range(NCORES):
        sl = slice(s * NS, (s + 1) * NS)
        o = res.results[s]["hc_out"]
        hv_out[sl] = o[0:NODE_H].T
        cv_out[sl] = o[NODE_H:].T
    return (hv_out, cv_out), res


def kernel(**inputs):
    out, _ = run(inputs, trace=False)
    return out



# revision 2
# speedup vs baseline: 3.8871x; 3.8871x over previous
"""Trainium2 Bass kernel for nn_NodeRNN (masked single-step LSTM over N nodes).

Strategy: the reference only *computes* on active rows (ts_mask==1, ~50%) and
passes old state through elsewhere. So the host gathers the active rows,
packs them (feature-major, bf16) into one contiguous DRAM image per core, the
device runs a dense unmasked LSTM step on the gathered rows, and the host
scatters results back (inactive rows are exact f32 passthrough). This halves
HBM traffic twice: active-only rows (~2x) and bf16 (2x).

Device (per core, CAP_PC=17408 gathered rows = 17 blocks x 1024):
  per block t one unified in-DMA [128, 6*1024] bf16 (12KB/partition runs):
    cols [xt c0 | c1 | c2 | c3 | hv | cv], xt chunk c = features 128c..128c+128
    of concat(hvv, Hv) (feature-major).
  x.T = relu(W @ feats + bias): 4 chunk matmuls (zero-padded lhsT) + one
    [3,128] aux matmul (rows x0,x1,1.0) that folds W_pos AND both biases, so
    relu is a pure DVE tensor_scalar_max out of PSUM.                 (PE+DVE)
  gates j: W_hh.T_j @ hv + W_ih.T_j @ x -> sigmoid/tanh (+gate bias)  (PE+ACT)
  c = f*cv + i*g; h = o*tanh(c) as bf16 tensor_tensor ops (2x DVE rate);
    tanh + h + output DMA batched per 4-block superblock (4096 cols). (DVE+ACT)
All matmuls bf16 (FWL weight loads, f32 PSUM accumulate).
"""
import sys

sys.path.insert(0, "/opt/trn_rl_repo")

import ml_dtypes
import numpy as np

import concourse.bacc as bacc
import concourse.tile as tile
from concourse import mybir
from concourse.bass_utils import run_bass_kernel_spmd

f32 = mybir.dt.float32
bf16 = mybir.dt.bfloat16
AF = mybir.ActivationFunctionType
ALU = mybir.AluOpType
nbf16 = ml_dtypes.bfloat16

N = 262144
NCORES = 8
TB = 1024                 # nodes per block
NBLK = 17                 # blocks per core
CAP_PC = NBLK * TB        # 17408 gathered rows per core
CAP = CAP_PC * NCORES     # 139264 total capacity (active ~131072, +32 sigma)
SUPERS = [4, 4, 4, 4, 1]  # blocks per superblock (output/tanh granularity)
EMBED = 64
EDGE_H = 256
NODE_H = 128

# cst block layout: [128, CF] bf16, free-dim offsets
CO_WHID = 0               # 4 chunks x 128 cols; cols 64:128 of chunk c = W_hid.T chunk
CO_WIH = 512              # W_ih.T [128, 512]
CO_WHH = 1024             # W_hh.T [128, 512]
CO_BG = 1536              # (b_ih + b_hh) as [128, 4], col j = gate chunk j
CO_WP = 1540              # [3, 128]: rows 0:2 = [W_pos.T | 0], row 2 = [b_pos | b_hid]
CF = 1668

IN_COLS = 6 * TB          # unified block: xt c0..c3, hv, cv
GATE_FUNCS = [AF.Sigmoid, AF.Sigmoid, AF.Tanh, AF.Sigmoid]  # i, f, g, o

_cached = {}


def build_nc():
    nc = bacc.Bacc(target_bir_lowering=False)
    blk_d = nc.dram_tensor("blk", [128, NBLK * IN_COLS], bf16, kind="ExternalInput")
    aux_d = nc.dram_tensor("aux", [3, CAP_PC], bf16, kind="ExternalInput")
    cst_d = nc.dram_tensor("cst", [128, CF], bf16, kind="ExternalInput")
    out_d = nc.dram_tensor("hc_out", [128, 2 * CAP_PC], bf16, kind="ExternalOutput")

    sup_of = []               # block -> (super idx, kb within super)
    for s, w in enumerate(SUPERS):
        for kb in range(w):
            sup_of.append((s, kb))
    sup_start = np.cumsum([0] + SUPERS)

    with tile.TileContext(nc) as tc:
        with (
            tc.tile_pool(name="const", bufs=1) as cpool,
            tc.tile_pool(name="inp", bufs=4) as inpp,
            tc.tile_pool(name="aux", bufs=4) as auxp,
            tc.tile_pool(name="xsb", bufs=3) as xsbp,
            tc.tile_pool(name="gact", bufs=6) as gactp,
            tc.tile_pool(name="tmp", bufs=4) as tmpp,
            tc.tile_pool(name="csb", bufs=2) as csbp,
            tc.tile_pool(name="osb", bufs=2) as osbp,
            tc.tile_pool(name="hsb", bufs=2) as hsbp,
            tc.tile_pool(name="ps_x", bufs=2, space="PSUM") as psx,
            tc.tile_pool(name="ps_g", bufs=2, space="PSUM") as psg,
        ):
            cst = cpool.tile([128, CF], bf16)
            nc.sync.dma_start(cst[:], cst_d[:])

            # warmup matmul absorbs the cst DMA wait on the PE
            warm = psx.tile([64, 512], f32, tag="x")
            nc.tensor.matmul(warm[:], cst[0:2, CO_WP:CO_WP + 64],
                             cst[0:2, 0:512], start=True, stop=True)

            stash = {}
            sup_state = {}

            def stage_a(t):
                it = inpp.tile([128, IN_COLS], bf16, tag="in")
                nc.sync.dma_start(it[:], blk_d[:, t * IN_COLS:(t + 1) * IN_COLS])
                at = auxp.tile([3, TB], bf16, tag="aux")
                nc.sync.dma_start(at[:], aux_d[:, t * TB:(t + 1) * TB])

                x_ps = psx.tile([128, TB], f32, tag="x")
                for k in range(2):
                    ksl = slice(k * 512, (k + 1) * 512)
                    for c in range(4):
                        nc.tensor.matmul(
                            x_ps[:, ksl],
                            cst[:, CO_WHID + 128 * c:CO_WHID + 128 * (c + 1)],
                            it[:, c * TB + k * 512:c * TB + (k + 1) * 512],
                            start=(c == 0), stop=False, skip_group_check=True)
                    # [3,128] lhsT: W_pos into partitions 0:64, b_pos/b_hid via
                    # the all-ones aux row -> bias is folded, relu needs no bias
                    nc.tensor.matmul(x_ps[:, ksl], cst[0:3, CO_WP:CO_WP + 128],
                                     at[:, ksl], start=False, stop=True,
                                     skip_group_check=True)
                x_sb = xsbp.tile([128, TB], bf16, tag="xsb")
                nc.vector.tensor_scalar_max(x_sb[:], x_ps[:], 0.0)
                stash[t] = (it, x_sb)

            def stage_b(t):
                it, x_sb = stash.pop(t)
                hv = it[:, 4 * TB:5 * TB]
                cv = it[:, 5 * TB:6 * TB]
                s, kb = sup_of[t]
                if kb == 0:
                    w = SUPERS[s] * TB
                    c_sb = csbp.tile([128, w], bf16, tag="csb")
                    o_sb = osbp.tile([128, w], bf16, tag="osb")
                    sup_state[s] = (c_sb, o_sb)
                c_sb, o_sb = sup_state[s]
                bsl = slice(kb * TB, (kb + 1) * TB)

                gact = []
                for j in range(4):
                    gp = psg.tile([128, TB], f32, tag="g")
                    for k in range(2):
                        ksl = slice(k * 512, (k + 1) * 512)
                        nc.tensor.matmul(
                            gp[:, ksl],
                            cst[:, CO_WHH + 128 * j:CO_WHH + 128 * (j + 1)],
                            hv[:, ksl], start=True, stop=False)
                        nc.tensor.matmul(
                            gp[:, ksl],
                            cst[:, CO_WIH + 128 * j:CO_WIH + 128 * (j + 1)],
                            x_sb[:, ksl], start=False, stop=True)
                    bias = cst[:, CO_BG + j:CO_BG + j + 1]
                    if j == 3:  # o-gate straight into the superblock buffer
                        nc.scalar.activation(o_sb[:, bsl], gp[:], GATE_FUNCS[j],
                                             bias=bias)
                    else:
                        ga = gactp.tile([128, TB], bf16, tag="ga")
                        gact.append(ga)
                        nc.scalar.activation(ga[:], gp[:], GATE_FUNCS[j], bias=bias)
                i_s, f_s, g_t = gact

                t1 = tmpp.tile([128, TB], bf16, tag="t1")
                t2 = tmpp.tile([128, TB], bf16, tag="t2")
                nc.vector.tensor_mul(t1[:], f_s[:], cv)
                nc.vector.tensor_mul(t2[:], i_s[:], g_t[:])
                nc.vector.tensor_add(c_sb[:, bsl], t1[:], t2[:])

                if kb == SUPERS[s] - 1:
                    w = SUPERS[s] * TB
                    th = tmpp.tile([128, w], bf16, tag="th", bufs=2)
                    nc.scalar.activation(th[:], c_sb[:], AF.Tanh)
                    h_sb = hsbp.tile([128, w], bf16, tag="hsb")
                    nc.vector.tensor_mul(h_sb[:], o_sb[:], th[:])
                    so = int(sup_start[s]) * 2 * TB
                    nc.sync.dma_start(out_d[:, so:so + w], h_sb[:])
                    nc.sync.dma_start(out_d[:, so + w:so + 2 * w], c_sb[:])

            for t in range(NBLK + 1):
                if t < NBLK:
                    stage_a(t)
                if t >= 1:
                    stage_b(t - 1)

    nc.finalize()
    return nc


def _pack_cst(W_pos, b_pos, W_hid, b_hid, W_ih, b_ih, W_hh, b_hh):
    cst = np.zeros((128, CF), dtype=np.float32)
    whid_t = np.ascontiguousarray(W_hid.T)          # [512, 64]
    for c in range(4):
        cst[:, CO_WHID + 128 * c + 64:CO_WHID + 128 * (c + 1)] = \
            whid_t[128 * c:128 * (c + 1)]
    cst[:, CO_WIH:CO_WIH + 512] = W_ih.T            # [128, 512]
    cst[:, CO_WHH:CO_WHH + 512] = W_hh.T
    bg = b_ih + b_hh
    cst[:, CO_BG:CO_BG + 4] = bg.reshape(4, 128).T
    cst[0:2, CO_WP:CO_WP + 64] = W_pos.T            # [2, 64]
    cst[2, CO_WP:CO_WP + 64] = b_pos
    cst[2, CO_WP + 64:CO_WP + 128] = b_hid
    return cst.astype(nbf16)


def _stage_chunk(idxc, Hv_t, hvv_t, xv_t, hv_tm1, cv_tm1, cst):
    """Gather rows idxc (padded to CAP), pack per-core bf16 DRAM images."""
    npad = CAP - len(idxc)
    ic = np.concatenate([idxc, np.zeros(npad, dtype=idxc.dtype)]) if npad else idxc

    hvv_g = hvv_t[ic].astype(nbf16)                 # [CAP, 256]
    Hv_g = Hv_t[ic].astype(nbf16)
    hv_g = hv_tm1[ic].astype(nbf16)                 # [CAP, 128]
    cv_g = cv_tm1[ic].astype(nbf16)
    aux_g = np.empty((3, CAP), dtype=nbf16)
    aux_g[0:2] = xv_t[ic].T
    aux_g[2] = np.ones(CAP, dtype=nbf16)

    in_maps = []
    for s in range(NCORES):
        sl = slice(s * CAP_PC, (s + 1) * CAP_PC)
        XT = np.empty((512, CAP_PC), dtype=nbf16)   # feature-major
        XT[0:256] = hvv_g[sl].T
        XT[256:] = Hv_g[sl].T
        blk = np.empty((128, NBLK, 6, TB), dtype=nbf16)
        blk[:, :, 0:4, :] = XT.reshape(4, 128, NBLK, TB).transpose(1, 2, 0, 3)
        blk[:, :, 4, :] = hv_g[sl].T.reshape(128, NBLK, TB)
        blk[:, :, 5, :] = cv_g[sl].T.reshape(128, NBLK, TB)
        in_maps.append(dict(blk=blk.reshape(128, NBLK * IN_COLS),
                            aux=np.ascontiguousarray(aux_g[:, sl]),
                            cst=cst))
    return in_maps


def _unpack_chunk(results):
    """Per-core device outputs -> [rows, 128] f32 h and c in gathered order."""
    sup_start = np.cumsum([0] + SUPERS)
    h_all = np.empty((NCORES * CAP_PC, NODE_H), dtype=np.float32)
    c_all = np.empty((NCORES * CAP_PC, NODE_H), dtype=np.float32)
    for s in range(NCORES):
        o = np.asarray(results[s]["hc_out"])        # [128, 2*CAP_PC] bf16
        r0 = s * CAP_PC
        for su, wblk in enumerate(SUPERS):
            w = wblk * TB
            so = int(sup_start[su]) * 2 * TB
            n0 = r0 + int(sup_start[su]) * TB
            h_all[n0:n0 + w] = o[:, so:so + w].T.astype(np.float32)
            c_all[n0:n0 + w] = o[:, so + w:so + 2 * w].T.astype(np.float32)
    return h_all, c_all


def run(inputs, trace=False, tmpdir=None):
    """Stage, run on 8 cores, unstage. Returns ((hv_t, cv_t), BassKernelResults)."""
    inputs = {k: np.asarray(v) for k, v in inputs.items()}
    cst = _pack_cst(inputs["W_pos"], inputs["b_pos"], inputs["W_hid"],
                    inputs["b_hid"], inputs["W_ih"], inputs["b_ih"],
                    inputs["W_hh"], inputs["b_hh"])
    idx = np.flatnonzero(inputs["ts_mask"][:, 0] == 1)

    hv_out = inputs["hv_tm1"].astype(np.float32, copy=True)
    cv_out = inputs["cv_tm1"].astype(np.float32, copy=True)

    if "nc" not in _cached:
        _cached["nc"] = build_nc()

    res = None
    for c0 in range(0, max(len(idx), 1), CAP):
        idxc = idx[c0:c0 + CAP]
        in_maps = _stage_chunk(idxc, inputs["Hv_t"], inputs["hvv_t"],
                               inputs["xv_t"], inputs["hv_tm1"],
                               inputs["cv_tm1"], cst)
        res = run_bass_kernel_spmd(_cached["nc"], in_maps,
                                   core_ids=list(range(NCORES)),
                                   trace=trace, tmpdir=tmpdir)
        if len(idxc):
            h_all, c_all = _unpack_chunk(res.results)
            hv_out[idxc] = h_all[:len(idxc)]
            cv_out[idxc] = c_all[:len(idxc)]
    return (hv_out, cv_out), res


def kernel(**inputs):
    out, _ = run(inputs, trace=False)
    return out
